# revision 1
# baseline (speedup 1.0000x reference)
"""AdaPool3d Trainium2 kernel — 8-core data parallel.

x [4,64,16,112,112] f32, beta [8,56,56] f32 -> out [4,64,8,56,56] f32.
256 (b,c) images sharded 32/core. Per image [16,112,112]:
  SBUF layout X [128, 1568] bf16 (DMA-cast): partition p=(kd,od,oh3),
  free f = ohp*224 + kh*112 + (2*ow+kw);  d=2*od+kd, h=16*ohp+2*oh3+kh.
Window sums via PE pooling matmuls (4 strided rhs accumulated), Dice
reciprocal via one fused custom-DVE op, softmax exps on ACT.
"""

import os
import numpy as np

_NCORES = 8
_IMGS = 32          # images per core
_D, _H, _W = 16, 112, 112
_OD, _OH, _OW = 8, 56, 56
_OHP, _OH3 = 7, 8   # oh = 8*ohp + oh3
_FD = 1568          # per-partition free elems = 7*224
_NW = 392           # windows per q-group = 7*56

_cache = {}


def _register_op(name, spec):
    from concourse.dve_spec import lower, _has_src1
    from concourse import dve_ops
    from concourse.dve_uop import DveOpSpec

    for op in dve_ops.OPS:
        if op.name == name:
            return op
    row = dve_ops._CUSTOM_DVE_ROW_BASE + len(dve_ops.OPS)
    assert row < 0x20
    dve_ops._SUB_OPCODE_FOR_NAME[name] = row
    shas = {}
    for ver in ("v3", "v4"):
        try:
            uops = lower(spec, ver=ver)
            shas[ver] = DveOpSpec(
                name=name, opcode=row, uops=uops, rd1_en=_has_src1(spec)
            ).sha(ver)
        except Exception:
            pass
    op = dve_ops.DveOp(name, spec, subdim=False, uops_sha=shas)
    dve_ops.OPS.append(op)
    dve_ops.CUSTOM_DVE_SPECS[name] = spec
    return op


def _register_custom_ops():
    """DICE: u*approx(1/(1+u^2)); RECIP1_EPS: approx(1/(x+eps)) 1-NR;
    DIV1: num*approx(1/den) 1-NR."""
    from concourse.dve_spec import Spec, Src0, Src1, Bin, AluOp, sq, One, C0, C1, C2

    def _r1(z, s0, s1):
        nb = (~z.view(np.int32)).view(np.float32)
        y0 = nb * np.float32(s0)
        return y0 * (np.float32(s1) - z * y0)

    _z = sq(Src0) + One
    _nb = Bin(AluOp.BITWISE_NOT, _z, _z)
    _y0 = _nb * C0
    _y1 = _y0 * (C1 - _z * _y0)
    dice = _register_op(
        "DICE_ANT",
        Spec(
            body=Src0 * _y1,
            reference=lambda in0, in1, s0, s1, imm2: in0
            * _r1((1.0 + in0.astype(np.float32) ** 2), s0, s1),
        ),
    )

    _ze = Src0 + C2
    _nbe = Bin(AluOp.BITWISE_NOT, _ze, _ze)
    _y0e = _nbe * C0
    _y1e = _y0e * (C1 - _ze * _y0e)
    recip1 = _register_op(
        "RECIP1_EPS_ANT",
        Spec(
            body=_y1e,
            reference=lambda in0, in1, s0, s1, imm2: _r1(
                in0.astype(np.float32) + np.float32(imm2), s0, s1
            ),
        ),
    )

    _nbd = Bin(AluOp.BITWISE_NOT, Src1, Src1)
    _y0d = _nbd * C0
    _y1d = _y0d * (C1 - Src1 * _y0d)
    div1 = _register_op(
        "DIV1_ANT",
        Spec(
            body=Src0 * _y1d,
            reference=lambda in0, in1, s0, s1, imm2: in0
            * _r1(in1.astype(np.float32), s0, s1),
        ),
    )
    return dice, recip1, div1


_C0, _C1 = -0.23549792, 2.0017324


def _build():
    if "nc" in _cache:
        return _cache["nc"]
    import concourse.bass as bass
    import concourse.bacc as bacc
    import concourse.mybir as mybir
    from concourse.tile import TileContext
    from contextlib import ExitStack

    DICE, RECIP1, DIV1 = _register_custom_ops()
    f32, bf16 = mybir.dt.float32, mybir.dt.bfloat16
    AF = mybir.ActivationFunctionType
    MUL, ADD, SUB = (
        mybir.AluOpType.mult,
        mybir.AluOpType.add,
        mybir.AluOpType.subtract,
    )

    nc = bacc.Bacc(None, target_bir_lowering=False, debug=False)
    # host pre-rearranged: x [img, p=(kd,od,oh3), f=(ohp,kh,w)]
    x_d = nc.dram_tensor("x", [_IMGS, 128, _FD], f32, kind="ExternalInput")
    beta_d = nc.dram_tensor("beta", [64, _NW], f32, kind="ExternalInput")
    lhs_d = nc.dram_tensor("lhs", [128, 192], f32, kind="ExternalInput")
    out_d = nc.dram_tensor("out", [_IMGS, 64, _NW], f32, kind="ExternalOutput")

    x_ap = x_d.ap()
    out_ap = out_d.ap()
    beta_v = beta_d.ap()

    with TileContext(nc) as tc, ExitStack() as ctx:
        const = ctx.enter_context(tc.tile_pool(name="const", bufs=1))
        xin = ctx.enter_context(tc.tile_pool(name="xin", bufs=4))
        big = ctx.enter_context(tc.tile_pool(name="big", bufs=3))
        sm = ctx.enter_context(tc.tile_pool(name="sm", bufs=3))
        ps = ctx.enter_context(tc.tile_pool(name="ps", bufs=1, space="PSUM"))
        pss = ctx.enter_context(tc.tile_pool(name="pss", bufs=1, space="PSUM"))

        # Constants: lhsT (avg cols 0:128 @0.125-dup, sum cols 128:192 @1.0)
        lhs_t = const.tile([128, 192], bf16, name="lhsT")
        nc.gpsimd.dma_start(out=lhs_t[:], in_=lhs_d.ap())
        lhsT_avg = lhs_t[:, 0:128]
        lhsT_sum = lhs_t[:, 128:192]
        beta_t = const.tile([64, _NW], f32, name="betat")
        nc.sync.dma_start(out=beta_t[:], in_=beta_v)

        for i in range(_IMGS):
            # ---- load image (cast f32->bf16); free = (ohp, ow, kh, kw)
            X = xin.tile([128, _FD], bf16, tag="X")
            nc.gpsimd.dma_start(out=X[:], in_=x_ap[i])
            Xf = X[:]
            X4 = X[:].rearrange("p (ab c) -> p ab c", c=4)  # [128, 392, 4]

            # ---- avg pool: 4 strided matmuls -> PSUM [128, 392] (kd-dup)
            pA = ps.tile([128, _NW], f32, tag="pA")
            for j in range(4):
                nc.tensor.matmul(
                    pA[:], lhsT_avg, X4[:, :, j],
                    start=(j == 0), stop=(j == 3),
                )

            # ---- recipA (eps inside op so zero window sums don't NaN)
            rA = sm.tile([128, _NW], f32, tag="rA")
            nc.vector._custom_dve(
                RECIP1, out=rA[:], in0=pA[:], s0=_C0, s1=_C1, imm2=1e-12
            )

            # ---- u = X * broadcast(rA) ; dsc = DICE(u)
            rA_b = (
                rA[:]
                .rearrange("p (ab u) -> p ab u", u=1)
                .broadcast_to([128, _NW, 4])
            )
            U = big.tile([128, _FD], bf16, tag="U")
            Uv = U[:].rearrange("p (ab c) -> p ab c", c=4)
            nc.vector.tensor_tensor(Uv, X4, rA_b, op=MUL)
            DS = big.tile([128, _FD], bf16, tag="DS")
            nc.vector._custom_dve(DICE, out=DS[:], in0=U[:], s0=_C0, s1=_C1)
            E = big.tile([128, _FD], bf16, tag="E")
            nc.scalar.activation(E[:], Xf, AF.Exp)
            F = big.tile([128, _FD], bf16, tag="F")
            nc.scalar.activation(F[:], DS[:], AF.Exp, scale=2.0)

            # ---- products
            M_ = big.tile([128, _FD], bf16, tag="M")
            nc.vector.tensor_tensor(M_[:], E[:], Xf, op=MUL)
            FX = big.tile([128, _FD], bf16, tag="FX")
            nc.vector.tensor_tensor(FX[:], F[:], Xf, op=MUL)

            # ---- window sums of e, m, f, fx -> PSUM [64, 392] each
            psums = {}
            for nm, T in (("e", E), ("m", M_), ("f", F), ("x", FX)):
                pT = pss.tile([64, _NW], f32, tag="p" + nm)
                Tv = T[:].rearrange("p (ab c) -> p ab c", c=4)
                for j in range(4):
                    nc.tensor.matmul(
                        pT[:], lhsT_sum, Tv[:, :, j],
                        start=(j == 0), stop=(j == 3),
                    )
                psums[nm] = pT

            # ---- combine: out = em + beta*(edscw - em)
            # (one PSUM read per instruction: denominators go to SBUF first)
            se_s = sm.tile([64, _NW], f32, tag="se_s")
            nc.vector.tensor_copy(se_s[:], psums["e"][:])
            sf_s = sm.tile([64, _NW], f32, tag="sf_s")
            nc.vector.tensor_copy(sf_s[:], psums["f"][:])
            em = sm.tile([64, _NW], f32, tag="em")
            nc.vector._custom_dve(
                DIV1, out=em[:], in0=psums["m"][:], in1=se_s[:],
                s0=_C0, s1=_C1,
            )
            ed = sm.tile([64, _NW], f32, tag="ed")
            nc.vector._custom_dve(
                DIV1, out=ed[:], in0=psums["x"][:], in1=sf_s[:],
                s0=_C0, s1=_C1,
            )
            dd = sm.tile([64, _NW], f32, tag="dd")
            nc.gpsimd.tensor_tensor(dd[:], ed[:], em[:], op=SUB)
            tt = sm.tile([64, _NW], f32, tag="tt")
            nc.gpsimd.tensor_tensor(tt[:], beta_t[:], dd[:], op=MUL)
            oc = sm.tile([64, _NW], f32, tag="oc")
            nc.gpsimd.tensor_tensor(oc[:], em[:], tt[:], op=ADD)

            # ---- store
            nc.sync.dma_start(out=out_ap[i], in_=oc[:])

    nc.finalize()
    _cache["nc"] = nc
    return nc


def _lhs_const():
    lhs = np.zeros((128, 192), np.float32)
    for p in range(128):
        q = p % 64
        lhs[p, q] = 0.125          # avg cols m in [0,128): m%64==q (dup)
        lhs[p, 64 + q] = 0.125
        lhs[p, 128 + q] = 1.0      # sum cols
    return lhs


def _prep_x(x, n):
    # [img, kd, od, oh3, ohp, ow, kh, kw] -> [img, 128, 1568]
    return np.ascontiguousarray(
        x.reshape(n, 8, 2, _OHP, _OH3, 2, 56, 2)
        .transpose(0, 2, 1, 4, 3, 6, 5, 7)
        .reshape(n, 128, _FD)
    )


def _prep_beta(beta):
    # beta [8,56,56] -> [q=(od,oh3), (ohp,ow)]
    return np.ascontiguousarray(
        beta.reshape(_OD, _OHP, _OH3, _OW).transpose(0, 2, 1, 3).reshape(64, _NW)
    )


def _unprep_out(outs, B, C):
    n = B * C
    return np.ascontiguousarray(
        outs.reshape(n, _OD, _OH3, _OHP, _OW)
        .transpose(0, 1, 3, 2, 4)
        .reshape(B, C, _OD, _OH, _OW)
    )


def kernel(**inputs):
    x = np.asarray(inputs["x"], dtype=np.float32)
    beta = np.asarray(inputs["beta"], dtype=np.float32)
    B, C = x.shape[0], x.shape[1]
    n = B * C
    x_r = _prep_x(x, n)
    beta_r = _prep_beta(beta)
    nc = _build()
    lhs = _lhs_const()
    in_maps = [
        {
            "x": np.ascontiguousarray(x_r[i * _IMGS : (i + 1) * _IMGS]),
            "beta": beta_r,
            "lhs": lhs,
        }
        for i in range(_NCORES)
    ]
    from concourse.bass_utils import run_bass_kernel_spmd

    res = run_bass_kernel_spmd(nc, in_maps, core_ids=list(range(_NCORES)))
    outs = np.stack([np.asarray(res.results[i]["out"]) for i in range(_NCORES)])
    return _unprep_out(outs, B, C)


if __name__ == "__main__":
    _build()
    print("build OK")



# revision 11
# speedup vs baseline: 1.2104x; 1.2104x over previous
"""AdaPool3d Trainium2 kernel — 8-core data parallel, v2.

x [4,64,16,112,112] f32, beta [8,56,56] f32 -> out [4,64,8,56,56] f32.
256 (b,c) images sharded 32/core, processed as 16 pairs/core.

Per image [16,112,112], SBUF layout X [128,1568] bf16 (DMA-cast of
SIG*x): partition p=(kd,od,oh3), free f=(kh,kw,ohp,ow); d=2*od+kd,
h=16*ohp+2*oh3+kh, w=2*ow+kw.  All window reductions are PE matmuls
against one constant lhsT (0.125 kd-pair selector, kd-dup for the avg,
col-halves 0:64/64:128 for even/odd image of a pair so the four
e/ex/f/fx sums of a pair share PSUM banks as [128,392]).

Math: u = x/avg via RECIP1 (NOT-trick+1NR) on the small [128,392] avg;
DS = DICEU(X, rA) = u*nb*(C1 - (1+u^2)*nb), nb=NOT(1+u^2) — an 8-op
fused DVE op whose NR seed scale is absorbed into C1=-8.5, making
DS = dsc/2 * G with G=18.0329.  The host pre-scales x by SIG=G/2 so a
single ACT exp over [X|DS] with scale=1/SIG yields [e^x | e^dsc].
em/edscw divisions fold beta and all scales into two host constants
invBE = SIG/(1-beta), invBF = SIG/beta; out = pM/(pE*invBE) +
pX/(pF*invBF) via two DIV1 ops and one bf16 add per pair.
"""

import numpy as np

_NCORES = 8
_IMGS = 32          # images per core
_PAIRS = 16
_OD, _OH, _OW = 8, 56, 56
_OHP, _OH3 = 7, 8   # oh = 8*ohp + oh3
_FD = 1568          # per-partition free elems = 4*392
_NW = 392           # windows per q-group = 7*56

_G = 18.032925      # DICEU gain (C1 = -8.5)
_SIG = _G / 2.0
_C1D = -8.5
_R0, _R1 = -0.23549792, 2.0017324  # NOT-trick + 1NR recip constants

_cache = {}


def _register_op(name, spec):
    from concourse.dve_spec import lower, _has_src1
    from concourse import dve_ops
    from concourse.dve_uop import DveOpSpec

    for op in dve_ops.OPS:
        if op.name == name:
            return op
    row = dve_ops._CUSTOM_DVE_ROW_BASE + len(dve_ops.OPS)
    assert row < 0x20
    dve_ops._SUB_OPCODE_FOR_NAME[name] = row
    shas = {}
    for ver in ("v3", "v4"):
        try:
            uops = lower(spec, ver=ver)
            shas[ver] = DveOpSpec(
                name=name, opcode=row, uops=uops, rd1_en=_has_src1(spec)
            ).sha(ver)
        except Exception:
            pass
    op = dve_ops.DveOp(name, spec, subdim=False, uops_sha=shas)
    dve_ops.OPS.append(op)
    dve_ops.CUSTOM_DVE_SPECS[name] = spec
    return op


def _np_not(z):
    return (~np.asarray(z, np.float32).view(np.int32)).view(np.float32)


def _np_r1(z, s0, s1):
    y0 = _np_not(z) * np.float32(s0)
    return y0 * (np.float32(s1) - z * y0)


def _register_custom_ops():
    """RECIP1_EPS: approx(1/(x+eps)), 1-NR; DICEU: fused u=x*r,
    dsc-like = u*nb*(C1-(1+u^2)*nb); DIV1: num*approx(1/den), 1-NR."""
    from concourse.dve_spec import Spec, Src0, Src1, Bin, AluOp, sq, One, C0, C1, C2

    _ze = Src0 + C2
    _nbe = Bin(AluOp.BITWISE_NOT, _ze, _ze)
    _y0e = _nbe * C0
    _y1e = _y0e * (C1 - _ze * _y0e)
    recip1 = _register_op(
        "RECIP1_EPS_ANT",
        Spec(
            body=_y1e,
            reference=lambda in0, in1, s0, s1, imm2: _np_r1(
                in0.astype(np.float32) + np.float32(imm2), s0, s1
            ),
        ),
    )

    _u = Src0 * Src1
    _z = sq(_u) + One
    _nb = Bin(AluOp.BITWISE_NOT, _z, _z)
    _q = _z * _nb
    _w = C1 - _q
    _m = _u * _nb

    def _diceu_ref(in0, in1, s0, s1, imm2):
        u = in0.astype(np.float32) * in1.astype(np.float32)
        z = (1.0 + u * u).astype(np.float32)
        nb = _np_not(z)
        return (u * nb).astype(np.float32) * (
            np.float32(s1) - (z * nb).astype(np.float32)
        )

    diceu = _register_op("DICEU_ANT", Spec(body=_m * _w, reference=_diceu_ref))

    _nbd = Bin(AluOp.BITWISE_NOT, Src1, Src1)
    _y0d = _nbd * C0
    _y1d = _y0d * (C1 - Src1 * _y0d)
    div1 = _register_op(
        "DIV1_ANT",
        Spec(
            body=Src0 * _y1d,
            reference=lambda in0, in1, s0, s1, imm2: in0
            * _np_r1(in1.astype(np.float32), s0, s1),
        ),
    )
    return recip1, diceu, div1


def _build():
    if "nc" in _cache:
        return _cache["nc"]
    import concourse.bass as bass
    import concourse.bacc as bacc
    import concourse.mybir as mybir
    from concourse.tile import TileContext
    from contextlib import ExitStack

    RECIP1, DICEU, DIV1 = _register_custom_ops()
    f32, bf16 = mybir.dt.float32, mybir.dt.bfloat16
    AF = mybir.ActivationFunctionType
    MUL, ADD = mybir.AluOpType.mult, mybir.AluOpType.add

    nc = bacc.Bacc(None, target_bir_lowering=False, debug=False)
    # host pre-rearranged: x [img, p=(kd,od,oh3), f=(kh,kw,ohp,ow)], x*SIG bf16
    x_d = nc.dram_tensor("x", [_IMGS, 128, _FD], bf16, kind="ExternalInput")
    lhs_d = nc.dram_tensor("lhs", [128, 128], bf16, kind="ExternalInput")
    ibe_d = nc.dram_tensor("invbe", [128, _NW], f32, kind="ExternalInput")
    ibf_d = nc.dram_tensor("invbf", [128, _NW], f32, kind="ExternalInput")
    out_d = nc.dram_tensor("out", [_PAIRS, 128, _NW], bf16, kind="ExternalOutput")

    x_ap = x_d.ap()
    out_ap = out_d.ap()

    with TileContext(nc) as tc, ExitStack() as ctx:
        const = ctx.enter_context(tc.tile_pool(name="const", bufs=1))
        xin = ctx.enter_context(tc.tile_pool(name="xin", bufs=3))
        ef = ctx.enter_context(tc.tile_pool(name="ef", bufs=3))
        mx = ctx.enter_context(tc.tile_pool(name="mx", bufs=3))
        sm = ctx.enter_context(tc.tile_pool(name="sm", bufs=2))
        psA = ctx.enter_context(tc.tile_pool(name="psA", bufs=2, space="PSUM"))
        pss = ctx.enter_context(tc.tile_pool(name="pss", bufs=1, space="PSUM"))

        lhs_t = const.tile([128, 128], bf16, name="lhsT")
        nc.sync.dma_start(out=lhs_t[:], in_=lhs_d.ap())
        ibe_t = const.tile([128, _NW], f32, name="ibe")
        nc.sync.dma_start(out=ibe_t[:], in_=ibe_d.ap())
        ibf_t = const.tile([128, _NW], f32, name="ibf")
        nc.sync.dma_start(out=ibf_t[:], in_=ibf_d.ap())

        for p in range(_PAIRS):
            psums = None
            for j in (0, 1):
                i = 2 * p + j
                # ---- load image (cast f32*SIG -> bf16); [X | DS] tile
                XX = xin.tile([128, 2 * _FD], bf16, tag="XX")
                nc.sync.dma_start(out=XX[:, 0:_FD], in_=x_ap[i])
                Xf = XX[:, 0:_FD]

                # ---- avg pool: 4 matmuls -> PSUM [128, 392] (kd-dup)
                pA = psA.tile([128, _NW], f32, tag="pA")
                for k in range(4):
                    nc.tensor.matmul(
                        pA[:], lhs_t[:], XX[:, k * _NW:(k + 1) * _NW],
                        start=(k == 0), stop=(k == 3),
                    )

                # ---- rA = approx 1/pA  (f32, [128,392])
                rA = sm.tile([128, _NW], f32, tag="rA")
                nc.vector._custom_dve(
                    RECIP1, out=rA[:], in0=pA[:], s0=_R0, s1=_R1, imm2=1e-12
                )

                # ---- DS = DICEU(X, bcast rA) into XX[:, FD:2FD]
                rA_b = (
                    rA[:]
                    .rearrange("q (s n) -> q s n", s=1)
                    .broadcast_to([128, 4, _NW])
                )
                nc.vector._custom_dve(
                    DICEU, out=XX[:, _FD:2 * _FD], in0=Xf, in1=rA_b, s1=_C1D
                )

                # ---- EF = exp(XX / SIG) = [e^x | e^dsc]
                EF = ef.tile([128, 2 * _FD], bf16, tag="EF")
                nc.scalar.activation(EF[:], XX[:], AF.Exp, scale=1.0 / _SIG)

                # ---- products: M = E*X (DVE), FX = F*X (GpSimd)
                MX = mx.tile([128, 2 * _FD], bf16, tag="MX")
                nc.vector.tensor_tensor(
                    MX[:, 0:_FD], EF[:, 0:_FD], Xf, op=MUL
                )
                nc.gpsimd.tensor_tensor(
                    MX[:, _FD:2 * _FD], EF[:, _FD:2 * _FD], Xf, op=MUL
                )

                # ---- window sums -> PSUM [128,392], pair-packed:
                # even image -> partitions 0:64, odd -> 64:128.
                if j == 0:
                    psums = {
                        nm: pss.tile(
                            [128, _NW], f32, tag="p" + nm, name="p" + nm
                        )
                        for nm in ("e", "m", "f", "x")
                    }
                lsl = lhs_t[:, 0:64] if j == 0 else lhs_t[:, 64:128]
                for nm, S in (
                    ("e", EF[:, 0:_FD]),
                    ("m", MX[:, 0:_FD]),
                    ("f", EF[:, _FD:2 * _FD]),
                    ("x", MX[:, _FD:2 * _FD]),
                ):
                    pT = psums[nm]
                    o = pT[0:64, :] if j == 0 else pT[64:128, :]
                    for k in range(4):
                        nc.tensor.matmul(
                            o, lsl, S[:, k * _NW:(k + 1) * _NW],
                            start=(k == 0),
                            stop=(k == 3),
                            skip_group_check=True,
                        )

            # ---- combine (per pair, [128,392] = 2 images)
            cE = sm.tile([128, _NW], f32, tag="cE")
            nc.vector.tensor_tensor(cE[:], psums["e"][:], ibe_t[:], op=MUL)
            cF = sm.tile([128, _NW], f32, tag="cF")
            nc.vector.tensor_tensor(cF[:], psums["f"][:], ibf_t[:], op=MUL)
            em = sm.tile([128, _NW], bf16, tag="em")
            nc.vector._custom_dve(
                DIV1, out=em[:], in0=psums["m"][:], in1=cE[:], s0=_R0, s1=_R1
            )
            ed = sm.tile([128, _NW], bf16, tag="ed")
            nc.vector._custom_dve(
                DIV1, out=ed[:], in0=psums["x"][:], in1=cF[:], s0=_R0, s1=_R1
            )
            oc = sm.tile([128, _NW], bf16, tag="oc")
            nc.vector.tensor_tensor(oc[:], em[:], ed[:], op=ADD)

            # ---- store (bf16 -> f32 cast in DMA)
            nc.sync.dma_start(out=out_ap[p], in_=oc[:])

    nc.finalize()
    _cache["nc"] = nc
    return nc


def _lhs_const():
    import ml_dtypes

    lhs = np.zeros((128, 128), np.float32)
    for q in range(64):
        lhs[q, q] = 0.125
        lhs[q, 64 + q] = 0.125
        lhs[64 + q, q] = 0.125
        lhs[64 + q, 64 + q] = 0.125
    return lhs.astype(ml_dtypes.bfloat16)


def _prep_x(x, n):
    import ml_dtypes

    # [img, od,kd, ohp,oh3,kh, ow,kw] -> [img, (kd,od,oh3), (kh,kw,ohp,ow)]
    return np.ascontiguousarray(
        (x * _SIG)
        .reshape(n, 8, 2, _OHP, _OH3, 2, 56, 2)
        .transpose(0, 2, 1, 4, 5, 7, 3, 6)
        .reshape(n, 128, _FD)
        .astype(ml_dtypes.bfloat16)
    )


def _prep_beta(beta):
    # beta [8,56,56] -> [q=(od,oh3), (ohp,ow)], dup to 128 partitions
    bq = (
        beta.reshape(_OD, _OHP, _OH3, _OW)
        .transpose(0, 2, 1, 3)
        .reshape(64, _NW)
        .astype(np.float32)
    )
    ibe = _SIG / np.maximum(1.0 - bq, 1e-7)
    ibf = _SIG / np.maximum(bq, 1e-7)
    return (
        np.ascontiguousarray(np.concatenate([ibe, ibe], axis=0)),
        np.ascontiguousarray(np.concatenate([ibf, ibf], axis=0)),
    )


def _unprep_out(outs, B, C):
    # outs [cores, PAIRS, 128, 392] -> [B, C, OD, OH, OW]
    n = B * C
    return np.ascontiguousarray(
        outs.reshape(n, _OD, _OH3, _OHP, _OW)
        .transpose(0, 1, 3, 2, 4)
        .reshape(B, C, _OD, _OH, _OW)
    )


def kernel(**inputs):
    x = np.asarray(inputs["x"], dtype=np.float32)
    beta = np.asarray(inputs["beta"], dtype=np.float32)
    B, C = x.shape[0], x.shape[1]
    n = B * C
    x_r = _prep_x(x, n)
    ibe, ibf = _prep_beta(beta)
    nc = _build()
    lhs = _lhs_const()
    in_maps = [
        {
            "x": np.ascontiguousarray(x_r[i * _IMGS:(i + 1) * _IMGS]),
            "lhs": lhs,
            "invbe": ibe,
            "invbf": ibf,
        }
        for i in range(_NCORES)
    ]
    from concourse.bass_utils import run_bass_kernel_spmd

    res = run_bass_kernel_spmd(nc, in_maps, core_ids=list(range(_NCORES)))
    outs = np.stack(
        [np.asarray(res.results[i]["out"]).astype(np.float32) for i in range(_NCORES)]
    )
    return _unprep_out(outs, B, C)


if __name__ == "__main__":
    _build()
    print("build OK")


# revision 13
# speedup vs baseline: 1.6138x; 1.3333x over previous
"""AdaPool3d Trainium2 kernel — 8-core data parallel, v2.

x [4,64,16,112,112] f32, beta [8,56,56] f32 -> out [4,64,8,56,56] f32.
256 (b,c) images sharded 32/core, processed as 16 pairs/core.

Per image [16,112,112], SBUF layout X [128,1568] bf16 (DMA-cast of
SIG*x): partition p=(kd,od,oh3), free f=(kh,kw,ohp,ow); d=2*od+kd,
h=16*ohp+2*oh3+kh, w=2*ow+kw.  All window reductions are PE matmuls
against one constant lhsT (0.125 kd-pair selector, kd-dup for the avg,
col-halves 0:64/64:128 for even/odd image of a pair so the four
e/ex/f/fx sums of a pair share PSUM banks as [128,392]).

Math: u = x/avg via RECIP1 (NOT-trick+1NR) on the small [128,392] avg;
DS = DICEU(X, rA) = u*nb*(C1 - (1+u^2)*nb), nb=NOT(1+u^2) — an 8-op
fused DVE op whose NR seed scale is absorbed into C1=-8.5, making
DS = dsc/2 * G with G=18.0329.  The host pre-scales x by SIG=G/2 so a
single ACT exp over [X|DS] with scale=1/SIG yields [e^x | e^dsc].
em/edscw divisions fold beta and all scales into two host constants
invBE = SIG/(1-beta), invBF = SIG/beta; out = pM/(pE*invBE) +
pX/(pF*invBF) via two DIV1 ops and one bf16 add per pair.
"""

import numpy as np

_NCORES = 8
_IMGS = 32          # images per core
_PAIRS = 16
_OD, _OH, _OW = 8, 56, 56
_OHP, _OH3 = 7, 8   # oh = 8*ohp + oh3
_FD = 1568          # per-partition free elems = 4*392
_NW = 392           # windows per q-group = 7*56

_G = 18.032925      # DICEU gain (C1 = -8.5)
_SIG = _G / 2.0
_C1D = -8.5
_R0, _R1 = -0.23549792, 2.0017324  # NOT-trick + 1NR recip constants

_cache = {}


def _register_op(name, spec):
    from concourse.dve_spec import lower, _has_src1
    from concourse import dve_ops
    from concourse.dve_uop import DveOpSpec

    for op in dve_ops.OPS:
        if op.name == name:
            return op
    row = dve_ops._CUSTOM_DVE_ROW_BASE + len(dve_ops.OPS)
    assert row < 0x20
    dve_ops._SUB_OPCODE_FOR_NAME[name] = row
    shas = {}
    for ver in ("v3", "v4"):
        try:
            uops = lower(spec, ver=ver)
            shas[ver] = DveOpSpec(
                name=name, opcode=row, uops=uops, rd1_en=_has_src1(spec)
            ).sha(ver)
        except Exception:
            pass
    op = dve_ops.DveOp(name, spec, subdim=False, uops_sha=shas)
    dve_ops.OPS.append(op)
    dve_ops.CUSTOM_DVE_SPECS[name] = spec
    return op


def _np_not(z):
    return (~np.asarray(z, np.float32).view(np.int32)).view(np.float32)


def _np_r1(z, s0, s1):
    y0 = _np_not(z) * np.float32(s0)
    return y0 * (np.float32(s1) - z * y0)


def _register_custom_ops():
    """RECIP1_EPS: approx(1/(x+eps)), 1-NR; DICEU: fused u=x*r,
    dsc-like = u*nb*(C1-(1+u^2)*nb); DIV1: num*approx(1/den), 1-NR."""
    from concourse.dve_spec import Spec, Src0, Src1, Bin, AluOp, sq, One, C0, C1, C2

    _ze = Src0 + C2
    _nbe = Bin(AluOp.BITWISE_NOT, _ze, _ze)
    _y0e = _nbe * C0
    _y1e = _y0e * (C1 - _ze * _y0e)
    recip1 = _register_op(
        "RECIP1_EPS_ANT",
        Spec(
            body=_y1e,
            reference=lambda in0, in1, s0, s1, imm2: _np_r1(
                in0.astype(np.float32) + np.float32(imm2), s0, s1
            ),
        ),
    )

    _u = Src0 * Src1
    _z = sq(_u) + One
    _nb = Bin(AluOp.BITWISE_NOT, _z, _z)
    _q = _z * _nb
    _w = C1 - _q
    _m = _u * _nb

    def _diceu_ref(in0, in1, s0, s1, imm2):
        u = in0.astype(np.float32) * in1.astype(np.float32)
        z = (1.0 + u * u).astype(np.float32)
        nb = _np_not(z)
        return (u * nb).astype(np.float32) * (
            np.float32(s1) - (z * nb).astype(np.float32)
        )

    diceu = _register_op("DICEU_ANT", Spec(body=_m * _w, reference=_diceu_ref))

    _nbd = Bin(AluOp.BITWISE_NOT, Src1, Src1)
    _y0d = _nbd * C0
    _y1d = _y0d * (C1 - Src1 * _y0d)
    div1 = _register_op(
        "DIV1_ANT",
        Spec(
            body=Src0 * _y1d,
            reference=lambda in0, in1, s0, s1, imm2: in0
            * _np_r1(in1.astype(np.float32), s0, s1),
        ),
    )
    return recip1, diceu, div1


def _build():
    if "nc" in _cache:
        return _cache["nc"]
    import concourse.bass as bass
    import concourse.bacc as bacc
    import concourse.mybir as mybir
    from concourse.tile import TileContext
    from contextlib import ExitStack

    RECIP1, DICEU, DIV1 = _register_custom_ops()
    f32, bf16 = mybir.dt.float32, mybir.dt.bfloat16
    AF = mybir.ActivationFunctionType
    MUL, ADD = mybir.AluOpType.mult, mybir.AluOpType.add

    nc = bacc.Bacc(None, target_bir_lowering=False, debug=False)
    # host pre-rearranged: x [img, p=(kd,od,oh3), f=(kh,kw,ohp,ow)], x*SIG bf16
    x_d = nc.dram_tensor("x", [_IMGS, 128, _FD], bf16, kind="ExternalInput")
    lhs_d = nc.dram_tensor("lhs", [128, 128], bf16, kind="ExternalInput")
    ibe_d = nc.dram_tensor("invbe", [128, _NW], f32, kind="ExternalInput")
    ibf_d = nc.dram_tensor("invbf", [128, _NW], f32, kind="ExternalInput")
    out_d = nc.dram_tensor("out", [_PAIRS, 128, _NW], bf16, kind="ExternalOutput")

    x_ap = x_d.ap()
    out_ap = out_d.ap()

    with TileContext(nc) as tc, ExitStack() as ctx:
        const = ctx.enter_context(tc.tile_pool(name="const", bufs=1))
        xin = ctx.enter_context(tc.tile_pool(name="xin", bufs=3))
        ef = ctx.enter_context(tc.tile_pool(name="ef", bufs=3))
        mx = ctx.enter_context(tc.tile_pool(name="mx", bufs=3))
        sm = ctx.enter_context(tc.tile_pool(name="sm", bufs=2))
        psA = ctx.enter_context(tc.tile_pool(name="psA", bufs=2, space="PSUM"))
        pss = ctx.enter_context(tc.tile_pool(name="pss", bufs=1, space="PSUM"))

        lhs_t = const.tile([128, 128], bf16, name="lhsT")
        nc.sync.dma_start(out=lhs_t[:], in_=lhs_d.ap())
        ibef_t = const.tile([128, 2 * _NW], f32, name="ibef")
        nc.sync.dma_start(out=ibef_t[:, 0:_NW], in_=ibe_d.ap())
        nc.sync.dma_start(out=ibef_t[:, _NW:2 * _NW], in_=ibf_d.ap())

        def mm_pair(out2d, lsl, S, tag_pos):
            # window sum over (kh,kw,kd): 4 accumulating matmuls of N=392
            for k in range(4):
                nc.tensor.matmul(
                    out2d, lsl, S[:, k * _NW:(k + 1) * _NW],
                    start=(k == 0), stop=(k == 3),
                    skip_group_check=True,
                )

        for p in range(_PAIRS):
            pEF = pMX = None
            for j in (0, 1):
                i = 2 * p + j
                # ---- load image; [X | DS] tile
                XX = xin.tile([128, 2 * _FD], bf16, tag="XX")
                nc.sync.dma_start(out=XX[:, 0:_FD], in_=x_ap[i])
                Xf = XX[:, 0:_FD]

                # ---- avg pool -> PSUM [128, 392] (kd-dup)
                pA = psA.tile([128, _NW], f32, tag="pA")
                mm_pair(pA[:], lhs_t[:], Xf, None)

                # ---- rA = approx 1/pA  (f32, [128,392])
                rA = sm.tile([128, _NW], f32, tag="rA")
                nc.vector._custom_dve(
                    RECIP1, out=rA[:], in0=pA[:], s0=_R0, s1=_R1, imm2=1e-12
                )

                # ---- DS = DICEU(X, bcast rA) into XX[:, FD:2FD]
                rA_b = (
                    rA[:]
                    .rearrange("q (s n) -> q s n", s=1)
                    .broadcast_to([128, 4, _NW])
                )
                nc.vector._custom_dve(
                    DICEU, out=XX[:, _FD:2 * _FD], in0=Xf, in1=rA_b, s1=_C1D
                )

                # ---- EF = exp(XX / SIG) = [e^x | e^dsc]
                EF = ef.tile([128, 2 * _FD], bf16, tag="EF")
                nc.scalar.activation(EF[:], XX[:], AF.Exp, scale=1.0 / _SIG)

                # ---- products [M | FX] = EF * [X | X] in one DVE op
                MX = mx.tile([128, 2 * _FD], bf16, tag="MX")
                X2b = (
                    Xf.rearrange("q (s n) -> q s n", s=1)
                    .broadcast_to([128, 2, _FD])
                )
                nc.vector.tensor_tensor(MX[:], EF[:], X2b, op=MUL)

                # ---- window sums -> bank-paired PSUM [128, 2, 512]:
                # even image -> partitions 0:64, odd -> 64:128.
                if j == 0:
                    pEF = pss.tile([128, 2, 512], f32, tag="pEF", name="pEF")
                    pMX = pss.tile([128, 2, 512], f32, tag="pMX", name="pMX")
                lsl = lhs_t[:, 0:64] if j == 0 else lhs_t[:, 64:128]
                qs = slice(0, 64) if j == 0 else slice(64, 128)
                mm_pair(pEF[qs, 0, 0:_NW], lsl, EF[:, 0:_FD], None)
                mm_pair(pMX[qs, 0, 0:_NW], lsl, MX[:, 0:_FD], None)
                mm_pair(pEF[qs, 1, 0:_NW], lsl, EF[:, _FD:2 * _FD], None)
                mm_pair(pMX[qs, 1, 0:_NW], lsl, MX[:, _FD:2 * _FD], None)

            # ---- combine (per pair, [128, 2*392] = 2 images x {em, ed})
            cEF = sm.tile([128, 2 * _NW], f32, tag="cEF")
            nc.vector.tensor_tensor(
                cEF[:], pEF[:, :, 0:_NW], ibef_t[:], op=MUL
            )
            emed = sm.tile([128, 2 * _NW], bf16, tag="emed")
            nc.vector._custom_dve(
                DIV1, out=emed[:], in0=pMX[:, :, 0:_NW], in1=cEF[:],
                s0=_R0, s1=_R1,
            )
            oc = sm.tile([128, _NW], bf16, tag="oc")
            nc.vector.tensor_tensor(
                oc[:], emed[:, 0:_NW], emed[:, _NW:2 * _NW], op=ADD
            )

            # ---- store
            nc.sync.dma_start(out=out_ap[p], in_=oc[:])

    nc.finalize()
    _cache["nc"] = nc
    return nc


def _lhs_const():
    import ml_dtypes

    lhs = np.zeros((128, 128), np.float32)
    for q in range(64):
        lhs[q, q] = 0.125
        lhs[q, 64 + q] = 0.125
        lhs[64 + q, q] = 0.125
        lhs[64 + q, 64 + q] = 0.125
    return lhs.astype(ml_dtypes.bfloat16)


def _prep_x(x, n):
    import ml_dtypes

    # [img, od,kd, ohp,oh3,kh, ow,kw] -> [img, (kd,od,oh3), (kh,kw,ohp,ow)]
    return np.ascontiguousarray(
        (x * _SIG)
        .reshape(n, 8, 2, _OHP, _OH3, 2, 56, 2)
        .transpose(0, 2, 1, 4, 5, 7, 3, 6)
        .reshape(n, 128, _FD)
        .astype(ml_dtypes.bfloat16)
    )


def _prep_beta(beta):
    # beta [8,56,56] -> [q=(od,oh3), (ohp,ow)], dup to 128 partitions
    bq = (
        beta.reshape(_OD, _OHP, _OH3, _OW)
        .transpose(0, 2, 1, 3)
        .reshape(64, _NW)
        .astype(np.float32)
    )
    ibe = _SIG / np.maximum(1.0 - bq, 1e-7)
    ibf = _SIG / np.maximum(bq, 1e-7)
    return (
        np.ascontiguousarray(np.concatenate([ibe, ibe], axis=0)),
        np.ascontiguousarray(np.concatenate([ibf, ibf], axis=0)),
    )


def _unprep_out(outs, B, C):
    # outs [cores, PAIRS, 128, 392] -> [B, C, OD, OH, OW]
    n = B * C
    return np.ascontiguousarray(
        outs.reshape(n, _OD, _OH3, _OHP, _OW)
        .transpose(0, 1, 3, 2, 4)
        .reshape(B, C, _OD, _OH, _OW)
    )


def kernel(**inputs):
    x = np.asarray(inputs["x"], dtype=np.float32)
    beta = np.asarray(inputs["beta"], dtype=np.float32)
    B, C = x.shape[0], x.shape[1]
    n = B * C
    x_r = _prep_x(x, n)
    ibe, ibf = _prep_beta(beta)
    nc = _build()
    lhs = _lhs_const()
    in_maps = [
        {
            "x": np.ascontiguousarray(x_r[i * _IMGS:(i + 1) * _IMGS]),
            "lhs": lhs,
            "invbe": ibe,
            "invbf": ibf,
        }
        for i in range(_NCORES)
    ]
    from concourse.bass_utils import run_bass_kernel_spmd

    res = run_bass_kernel_spmd(nc, in_maps, core_ids=list(range(_NCORES)))
    outs = np.stack(
        [np.asarray(res.results[i]["out"]).astype(np.float32) for i in range(_NCORES)]
    )
    return _unprep_out(outs, B, C)


if __name__ == "__main__":
    _build()
    print("build OK")


# revision 18
# speedup vs baseline: 1.6654x; 1.0319x over previous
"""AdaPool3d Trainium2 kernel — 8-core data parallel, v2.

x [4,64,16,112,112] f32, beta [8,56,56] f32 -> out [4,64,8,56,56] f32.
256 (b,c) images sharded 32/core, processed as 16 pairs/core.

Per image [16,112,112], SBUF layout X [128,1568] bf16 (DMA-cast of
SIG*x): partition p=(kd,od,oh3), free f=(kh,kw,ohp,ow); d=2*od+kd,
h=16*ohp+2*oh3+kh, w=2*ow+kw.  All window reductions are PE matmuls
against one constant lhsT (0.125 kd-pair selector, kd-dup for the avg,
col-halves 0:64/64:128 for even/odd image of a pair so the four
e/ex/f/fx sums of a pair share PSUM banks as [128,392]).

Math: u = x/avg via RECIP1 (NOT-trick+1NR) on the small [128,392] avg;
DS = DICEU(X, rA) = u*nb*(C1 - (1+u^2)*nb), nb=NOT(1+u^2) — an 8-op
fused DVE op whose NR seed scale is absorbed into C1=-8.5, making
DS = dsc/2 * G with G=18.0329.  The host pre-scales x by SIG=G/2 so a
single ACT exp over [X|DS] with scale=1/SIG yields [e^x | e^dsc].
em/edscw divisions fold beta and all scales into two host constants
invBE = SIG/(1-beta), invBF = SIG/beta; out = pM/(pE*invBE) +
pX/(pF*invBF) via two DIV1 ops and one bf16 add per pair.
"""

import numpy as np

_NCORES = 8
_IMGS = 32          # images per core
_PAIRS = 16
_OD, _OH, _OW = 8, 56, 56
_OHP, _OH3 = 7, 8   # oh = 8*ohp + oh3
_FD = 1568          # per-partition free elems = 4*392
_NW = 392           # windows per q-group = 7*56

_G = 18.032925      # DICEU gain (C1 = -8.5)
_SIG = _G / 2.0
_C1D = -8.5
_R0, _R1 = -0.23549792, 2.0017324  # NOT-trick + 1NR recip constants

_cache = {}


def _register_op(name, spec):
    from concourse.dve_spec import lower, _has_src1
    from concourse import dve_ops
    from concourse.dve_uop import DveOpSpec

    for op in dve_ops.OPS:
        if op.name == name:
            return op
    row = dve_ops._CUSTOM_DVE_ROW_BASE + len(dve_ops.OPS)
    assert row < 0x20
    dve_ops._SUB_OPCODE_FOR_NAME[name] = row
    shas = {}
    for ver in ("v3", "v4"):
        try:
            uops = lower(spec, ver=ver)
            shas[ver] = DveOpSpec(
                name=name, opcode=row, uops=uops, rd1_en=_has_src1(spec)
            ).sha(ver)
        except Exception:
            pass
    op = dve_ops.DveOp(name, spec, subdim=False, uops_sha=shas)
    dve_ops.OPS.append(op)
    dve_ops.CUSTOM_DVE_SPECS[name] = spec
    return op


def _np_not(z):
    return (~np.asarray(z, np.float32).view(np.int32)).view(np.float32)


def _np_r1(z, s0, s1):
    y0 = _np_not(z) * np.float32(s0)
    return y0 * (np.float32(s1) - z * y0)


def _register_custom_ops():
    """RECIP1_EPS: approx(1/(x+eps)), 1-NR; DICEU: fused u=x*r,
    dsc-like = u*nb*(C1-(1+u^2)*nb); DIV1: num*approx(1/den), 1-NR."""
    from concourse.dve_spec import Spec, Src0, Src1, Bin, AluOp, sq, One, C0, C1, C2

    _ze = Src0 + C2
    _nbe = Bin(AluOp.BITWISE_NOT, _ze, _ze)
    _y0e = _nbe * C0
    _y1e = _y0e * (C1 - _ze * _y0e)
    recip1 = _register_op(
        "RECIP1_EPS_ANT",
        Spec(
            body=_y1e,
            reference=lambda in0, in1, s0, s1, imm2: _np_r1(
                in0.astype(np.float32) + np.float32(imm2), s0, s1
            ),
        ),
    )

    _u = Src0 * Src1
    _z = sq(_u) + One
    _nb = Bin(AluOp.BITWISE_NOT, _z, _z)
    _q = _z * _nb
    _w = C1 - _q
    _m = _u * _nb

    def _diceu_ref(in0, in1, s0, s1, imm2):
        u = in0.astype(np.float32) * in1.astype(np.float32)
        z = (1.0 + u * u).astype(np.float32)
        nb = _np_not(z)
        return (u * nb).astype(np.float32) * (
            np.float32(s1) - (z * nb).astype(np.float32)
        )

    diceu = _register_op("DICEU_ANT", Spec(body=_m * _w, reference=_diceu_ref))

    _nbd = Bin(AluOp.BITWISE_NOT, Src1, Src1)
    _y0d = _nbd * C0
    _y1d = _y0d * (C1 - Src1 * _y0d)
    div1 = _register_op(
        "DIV1_ANT",
        Spec(
            body=Src0 * _y1d,
            reference=lambda in0, in1, s0, s1, imm2: in0
            * _np_r1(in1.astype(np.float32), s0, s1),
        ),
    )
    return recip1, diceu, div1


def _build():
    if "nc" in _cache:
        return _cache["nc"]
    import concourse.bass as bass
    import concourse.bacc as bacc
    import concourse.mybir as mybir
    from concourse.tile import TileContext
    from contextlib import ExitStack

    RECIP1, DICEU, DIV1 = _register_custom_ops()
    f32, bf16 = mybir.dt.float32, mybir.dt.bfloat16
    AF = mybir.ActivationFunctionType
    MUL, ADD = mybir.AluOpType.mult, mybir.AluOpType.add

    nc = bacc.Bacc(None, target_bir_lowering=False, debug=False)
    # host pre-rearranged pairs: x [pair, p=(kd,od,oh3), f=(kh,kw,img,ohp,ow)]
    x_d = nc.dram_tensor("x", [_PAIRS, 128, 2 * _FD], bf16, kind="ExternalInput")
    lhs_d = nc.dram_tensor("lhs", [128, 128], bf16, kind="ExternalInput")
    ibe_d = nc.dram_tensor("invbe", [128, _NW], f32, kind="ExternalInput")
    ibf_d = nc.dram_tensor("invbf", [128, _NW], f32, kind="ExternalInput")
    out_d = nc.dram_tensor("out", [_PAIRS, 128, _NW], bf16, kind="ExternalOutput")

    x_ap = x_d.ap()
    out_ap = out_d.ap()

    with TileContext(nc) as tc, ExitStack() as ctx:
        const = ctx.enter_context(tc.tile_pool(name="const", bufs=1))
        xin = ctx.enter_context(tc.tile_pool(name="xin", bufs=3))
        ef = ctx.enter_context(tc.tile_pool(name="ef", bufs=3))
        mx = ctx.enter_context(tc.tile_pool(name="mx", bufs=3))
        sm = ctx.enter_context(tc.tile_pool(name="sm", bufs=2))
        psA = ctx.enter_context(tc.tile_pool(name="psA", bufs=1, space="PSUM"))
        pss = ctx.enter_context(tc.tile_pool(name="pss", bufs=1, space="PSUM"))

        lhs_t = const.tile([128, 128], bf16, name="lhsT")
        nc.sync.dma_start(out=lhs_t[:], in_=lhs_d.ap())
        ibef_t = const.tile([128, 2 * _NW], f32, name="ibef")
        nc.sync.dma_start(out=ibef_t[:, 0:_NW], in_=ibe_d.ap())
        nc.sync.dma_start(out=ibef_t[:, _NW:2 * _NW], in_=ibf_d.ap())

        for p in range(_PAIRS):
            # ---- load pair; [Xpair | DSpair] tile, f=(kh,kw,img,ohp,ow)
            XX = xin.tile([128, 4 * _FD], bf16, tag="XX")
            nc.sync.dma_start(out=XX[:, 0:2 * _FD], in_=x_ap[p])
            Xf = XX[:, 0:2 * _FD]

            # ---- avg pools -> bank-paired PSUM (kd-dup), img j in bank j
            pA2 = psA.tile([128, 2, 512], f32, tag="pA2", name="pA2")
            for j in (0, 1):
                for k in range(4):
                    nc.tensor.matmul(
                        pA2[:, j, 0:_NW], lhs_t[:],
                        XX[:, k * 2 * _NW + j * _NW:k * 2 * _NW + (j + 1) * _NW],
                        start=(k == 0), stop=(k == 3),
                        skip_group_check=True,
                    )

            # ---- rA2 = approx 1/pA2  (f32, [128, 784])
            rA2 = sm.tile([128, 2 * _NW], f32, tag="rA2")
            nc.vector._custom_dve(
                RECIP1, out=rA2[:], in0=pA2[:, :, 0:_NW],
                s0=_R0, s1=_R1, imm2=1e-12,
            )

            # ---- DS = DICEU(X, bcast rA2) into XX[:, 2FD:4FD]
            rA_b = (
                rA2[:]
                .rearrange("q (s n) -> q s n", s=1)
                .broadcast_to([128, 4, 2 * _NW])
            )
            nc.vector._custom_dve(
                DICEU, out=XX[:, 2 * _FD:4 * _FD], in0=Xf, in1=rA_b, s1=_C1D
            )

            # ---- EF = exp(XX / SIG) = [e^x | e^dsc] for the pair
            EF = ef.tile([128, 4 * _FD], bf16, tag="EF")
            nc.scalar.activation(EF[:], XX[:], AF.Exp, scale=1.0 / _SIG)

            # ---- products [M | FX] = EF * [X | X] in one DVE op
            MX = mx.tile([128, 4 * _FD], bf16, tag="MX")
            X2b = (
                Xf.rearrange("q (s n) -> q s n", s=1)
                .broadcast_to([128, 2, 2 * _FD])
            )
            nc.vector.tensor_tensor(MX[:], EF[:], X2b, op=MUL)

            # ---- window sums -> bank-paired PSUM [128, 2, 512]:
            # even image -> partitions 0:64, odd -> 64:128.
            pEF = pss.tile([128, 2, 512], f32, tag="pEF", name="pEF")
            pMX = pss.tile([128, 2, 512], f32, tag="pMX", name="pMX")
            for j in (0, 1):
                lsl = lhs_t[:, 0:64] if j == 0 else lhs_t[:, 64:128]
                qs = slice(0, 64) if j == 0 else slice(64, 128)
                for pT, base in (
                    (pEF, 0), (pMX, 0), (pEF, 2 * _FD), (pMX, 2 * _FD)
                ):
                    S = EF if pT is pEF else MX
                    b = 0 if base == 0 else 1
                    o = pT[qs, b, 0:_NW]
                    for k in range(4):
                        off = base + k * 2 * _NW + j * _NW
                        nc.tensor.matmul(
                            o, lsl, S[:, off:off + _NW],
                            start=(k == 0), stop=(k == 3),
                            skip_group_check=True,
                        )

            # ---- combine (per pair, [128, 2*392] = 2 images x {em, ed})
            cEF = sm.tile([128, 2 * _NW], f32, tag="cEF")
            nc.vector.tensor_tensor(
                cEF[:], pEF[:, :, 0:_NW], ibef_t[:], op=MUL
            )
            emed = sm.tile([128, 2 * _NW], bf16, tag="emed")
            nc.vector._custom_dve(
                DIV1, out=emed[:], in0=pMX[:, :, 0:_NW], in1=cEF[:],
                s0=_R0, s1=_R1,
            )
            oc = sm.tile([128, _NW], bf16, tag="oc")
            nc.vector.tensor_tensor(
                oc[:], emed[:, 0:_NW], emed[:, _NW:2 * _NW], op=ADD
            )

            # ---- store
            nc.sync.dma_start(out=out_ap[p], in_=oc[:])

    nc.finalize()
    _cache["nc"] = nc
    return nc


def _lhs_const():
    import ml_dtypes

    lhs = np.zeros((128, 128), np.float32)
    for q in range(64):
        lhs[q, q] = 0.125
        lhs[q, 64 + q] = 0.125
        lhs[64 + q, q] = 0.125
        lhs[64 + q, 64 + q] = 0.125
    return lhs.astype(ml_dtypes.bfloat16)


def _prep_x(x, n):
    import ml_dtypes

    # [img, od,kd, ohp,oh3,kh, ow,kw] -> [img, (kd,od,oh3), (kh,kw,ohp,ow)]
    return np.ascontiguousarray(
        (x * _SIG)
        .reshape(n, 8, 2, _OHP, _OH3, 2, 56, 2)
        .transpose(0, 2, 1, 4, 5, 7, 3, 6)
        .reshape(n, 128, _FD)
        .astype(ml_dtypes.bfloat16)
    )


def _prep_beta(beta):
    # beta [8,56,56] -> [q=(od,oh3), (ohp,ow)], dup to 128 partitions
    bq = (
        beta.reshape(_OD, _OHP, _OH3, _OW)
        .transpose(0, 2, 1, 3)
        .reshape(64, _NW)
        .astype(np.float32)
    )
    ibe = _SIG / np.maximum(1.0 - bq, 1e-7)
    ibf = _SIG / np.maximum(bq, 1e-7)
    return (
        np.ascontiguousarray(np.concatenate([ibe, ibe], axis=0)),
        np.ascontiguousarray(np.concatenate([ibf, ibf], axis=0)),
    )


def _unprep_out(outs, B, C):
    # outs [cores, PAIRS, 128, 392] -> [B, C, OD, OH, OW]
    n = B * C
    return np.ascontiguousarray(
        outs.reshape(n, _OD, _OH3, _OHP, _OW)
        .transpose(0, 1, 3, 2, 4)
        .reshape(B, C, _OD, _OH, _OW)
    )


def _pairify(x_core):
    # [32, 128, (khw4, 392)] -> [16, 128, (khw4, img2, 392)]
    return np.ascontiguousarray(
        x_core.reshape(_PAIRS, 2, 128, 4, _NW)
        .transpose(0, 2, 3, 1, 4)
        .reshape(_PAIRS, 128, 2 * _FD)
    )


def kernel(**inputs):
    x = np.asarray(inputs["x"], dtype=np.float32)
    beta = np.asarray(inputs["beta"], dtype=np.float32)
    B, C = x.shape[0], x.shape[1]
    n = B * C
    x_r = _prep_x(x, n)
    ibe, ibf = _prep_beta(beta)
    nc = _build()
    lhs = _lhs_const()
    in_maps = [
        {
            "x": _pairify(x_r[i * _IMGS:(i + 1) * _IMGS]),
            "lhs": lhs,
            "invbe": ibe,
            "invbf": ibf,
        }
        for i in range(_NCORES)
    ]
    from concourse.bass_utils import run_bass_kernel_spmd

    res = run_bass_kernel_spmd(nc, in_maps, core_ids=list(range(_NCORES)))
    outs = np.stack(
        [np.asarray(res.results[i]["out"]).astype(np.float32) for i in range(_NCORES)]
    )
    return _unprep_out(outs, B, C)


if __name__ == "__main__":
    _build()
    print("build OK")
